# revision 11
# baseline (speedup 1.0000x reference)
"""Multi-head self-attention (B=4, T=2048, D=1024, H=16) on 8 TRN2 NeuronCores.

Sharding: core c = 2*b + j computes batch b, heads j*8..j*8+7 (tensor-parallel
over heads), and a partial projection over its 512 attention-output columns.
The host sums the two partial projections per batch. No collectives.

v2 over the v1 baseline:
  - Scores use PE row tiling: a head pair's two K=64 matmuls run concurrently
    on row groups (0,0)/(64,0) instead of zero-padding Q to a 128 contraction.
    Halves the scores PE time; the zero-pad memsets are gone.
  - The whole kernel is emitted as one software-pipelined stream paced by the
    ScalarE exp wall (~257us): QKV chunks for pair p+1, V chunks, and the
    attn@V matmuls are interleaved between per-kt scores+exp steps, so the
    exp stream starts ~35us in (vs ~110us) and runs uninterrupted.
  - Softmax denominators (row 64 of the attn@V psum, a [1,1024] row) are
    reshaped to [64,16] via two tiny SBUF DMAs around the DVE reciprocal
    instead of a single-lane [1,1024] reciprocal (was 6.5us each, 105us
    total on DVE).
  - PSUM: scores ring 2x[128,1024] (4 banks) + po_A/po_B accumulators
    (4 banks); QKV/V/proj chains borrow the scores ring.
"""

import os

import numpy as np
import ml_dtypes

import concourse.mybir as mybir
from concourse import bacc
from concourse.tile import TileContext
from concourse.bass_utils import run_bass_kernel_spmd

B, T, D, H = 4, 2048, 1024, 16
HD = D // H
SCALE = HD**-0.5
P = 128
BF = mybir.dt.bfloat16
F32 = mybir.dt.float32
NBF = ml_dtypes.bfloat16

LAST_RESULT = None
_built = None


def _build():
    nc = bacc.Bacc("TRN2", target_bir_lowering=False, debug=False, num_devices=8)

    xT = nc.dram_tensor("xT", [D, T], BF, kind="ExternalInput")  # x[b].T
    wqkT = nc.dram_tensor("wqkT", [D, 1024], BF, kind="ExternalInput")  # (q|k).T shard
    wvT = nc.dram_tensor("wvT", [D, 512], BF, kind="ExternalInput")
    wpT = nc.dram_tensor("wpT", [512, D], BF, kind="ExternalInput")  # proj_w.T rows
    qkb = nc.dram_tensor("qkb", [1024], F32, kind="ExternalInput")
    vb = nc.dram_tensor("vb", [512], F32, kind="ExternalInput")
    pb = nc.dram_tensor("pb", [D], F32, kind="ExternalInput")
    yT = nc.dram_tensor("yT", [D, T], F32, kind="ExternalOutput")

    Exp = mybir.ActivationFunctionType.Exp
    mult = mybir.AluOpType.mult
    add = mybir.AluOpType.add

    with TileContext(nc) as tc:
        with (
            tc.tile_pool(name="pers", bufs=1) as pers,
            tc.tile_pool(name="small", bufs=1) as small,
        ):
            # ---- persistent tensors ----
            # K^T pair tiles: rows 0:64 even head's 64 features, 64:128 odd's
            kts = [pers.tile([P, T], BF, tag=f"kt{i}", name=f"kt{i}") for i in range(4)]
            # Q^T pair tiles, same feature packing (no zero padding: the scores
            # matmuls are row-group tiled at K=64)
            qps = [pers.tile([P, T], BF, tag=f"qp{i}", name=f"qp{i}") for i in range(4)]
            # normalized attention out, feature-major, per pair
            ats = [pers.tile([P, T], BF, tag=f"at{i}", name=f"at{i}") for i in range(4)]
            V4 = pers.tile([P, 16, 8, HD + 1], BF, tag="v4")  # [t-part, tt, head, 65]

            epool_cm = tc.tile_pool(name="epool", bufs=4)
            npool_cm = tc.tile_pool(name="npool", bufs=1)
            spool_cm = tc.tile_pool(name="spool", bufs=2, space="PSUM")
            opool_cm = tc.tile_pool(name="opool", bufs=2, space="PSUM")
            epool = epool_cm.__enter__()
            npool = npool_cm.__enter__()
            spool = spool_cm.__enter__()
            opool = opool_cm.__enter__()

            ph1_cm = tc.tile_pool(name="ph1", bufs=1)
            ph1 = ph1_cm.__enter__()
            xts = []
            wqks = []
            for dt in range(8):
                t_ = ph1.tile([P, T], BF, tag=f"xt{dt}")
                nc.sync.dma_start(t_[:], xT.ap()[dt * P : (dt + 1) * P, :])
                xts.append(t_)
                w_ = ph1.tile([P, 1024], BF, tag=f"wqk{dt}")
                nc.sync.dma_start(w_[:], wqkT.ap()[dt * P : (dt + 1) * P, :])
                wqks.append(w_)
            wvs = []
            for dt in range(8):
                w_ = ph1.tile([P, 512], BF, tag=f"wv{dt}")
                nc.sync.dma_start(w_[:], wvT.ap()[dt * P : (dt + 1) * P, :])
                wvs.append(w_)
            qkb_sb = small.tile([P, 8], F32, tag="qkb")
            nc.sync.dma_start(qkb_sb[:], qkb.rearrange("(o p) -> p o", p=P))
            vb_sb = small.tile([P, 512], F32, tag="vb")
            nc.sync.dma_start(vb_sb[:], vb.ap()[None, :].to_broadcast((P, 512)))
            pb_sb = small.tile([P, 8], F32, tag="pb")
            nc.sync.dma_start(pb_sb[:], pb.rearrange("(o p) -> p o", p=P))
            nc.vector.memset(V4[:, :, :, HD : HD + 1], 1.0)

            if True:
                # ---- emission helpers (each emits one PE chunk ~0.9-1.7us) ----

                def qkv_chunk(ft, tcc, half):
                    # one [128,512] column block of the qk projection:
                    # psum[feat, t] = wqkT.T @ xT, 8 accumulation matmuls
                    pq = spool.tile([P, 1024], F32, tag="ps", name="pq")
                    col = tcc * 1024 + half * 512
                    for dt in range(8):
                        nc.tensor.matmul(
                            pq[:, 0:512],
                            lhsT=wqks[dt][:, ft * P : (ft + 1) * P],
                            rhs=xts[dt][:, col : col + 512],
                            start=(dt == 0),
                            stop=(dt == 7),
                        )
                    dst = kts[ft - 4] if ft >= 4 else qps[ft]
                    nc.vector.tensor_scalar_add(
                        dst[:, col : col + 512],
                        pq[:, 0:512],
                        qkb_sb[:, ft : ft + 1],
                    )

                def v_chunk(tt, hh):
                    # V[t, feat] for head block hh (4 heads = 256 features)
                    pv = spool.tile([P, 1024], F32, tag="ps", name="pv")
                    fcol = hh * 256
                    for dt in range(8):
                        nc.tensor.matmul(
                            pv[:, 0:256],
                            lhsT=xts[dt][:, tt * P : (tt + 1) * P],
                            rhs=wvs[dt][:, fcol : fcol + 256],
                            start=(dt == 0),
                            stop=(dt == 7),
                        )
                    nc.vector.tensor_tensor(
                        V4[:, tt, hh * 4 : hh * 4 + 4, 0:HD],
                        pv[:, 0:256].rearrange("p (h e) -> p h e", e=HD),
                        vb_sb[:, fcol : fcol + 256].rearrange("p (h e) -> p h e", e=HD),
                        add,
                    )

                # ---- QK^T projection for pair 0 (ramp) ----
                for ft in (4, 0):
                    for tcc in range(2):
                        for half in range(2):
                            qkv_chunk(ft, tcc, half)

                # extras: per iteration, a list of chunk thunks drained one
                # per kt step (each ~<=1.7us of PE work)
                extras = [[] for _ in range(8)]
                # iter 0: V for pairs 0+1 (tt chunk k must land before the
                # lagged attn@V consumes V4[:, k], i.e. by step k+8)
                extras[0] = [lambda tt=tt: v_chunk(tt, 0) for tt in range(16)]
                # iter 1: qk projection for pair 1
                extras[1] = [
                    lambda a=a, b=b, c=c: qkv_chunk(a, b, c)
                    for a in (5, 1)
                    for b in range(2)
                    for c in range(2)
                ]
                # iter 2: V for pairs 2+3
                extras[2] = [lambda tt=tt: v_chunk(tt, 1) for tt in range(16)]
                # iter 3: qk for pair 2; iter 5: qk for pair 3
                extras[3] = [
                    lambda a=a, b=b, c=c: qkv_chunk(a, b, c)
                    for a in (6, 2)
                    for b in range(2)
                    for c in range(2)
                ]
                extras[5] = [
                    lambda a=a, b=b, c=c: qkv_chunk(a, b, c)
                    for a in (7, 3)
                    for b in range(2)
                    for c in range(2)
                ]

                def emit_norm(p, qcc, po_, rows):
                    # po_[0:64] are head outputs, po_[64] the exp-sum row.
                    # Reshape the [1,1024] denominator across 64 partitions for
                    # the reciprocal (a [1,n] DVE op runs on a single lane).
                    drow = npool.tile([1, 1024], F32, tag="drow")
                    nc.vector.tensor_scalar_add(drow[:], po_[HD : HD + 1, :], 0.0)
                    dsh = npool.tile([64, 16], F32, tag="dsh")
                    nc.sync.dma_start(
                        dsh[:], drow[:].rearrange("a (p j) -> a p j", p=64)
                    )
                    rsh = npool.tile([64, 16], F32, tag="rsh")
                    nc.vector.reciprocal(rsh[:], dsh[:])
                    rrow = npool.tile([1, 1024], F32, tag="rrow")
                    nc.sync.dma_start(
                        rrow[:].rearrange("a (p j) -> a p j", p=64), rsh[:]
                    )
                    rb = npool.tile([64, 1024], F32, tag="rb")
                    nc.gpsimd.partition_broadcast(rb[:], rrow[:])
                    nc.vector.tensor_tensor(
                        ats[p][rows, qcc * 1024 : (qcc + 1) * 1024],
                        po_[0:HD, :],
                        rb[:],
                        mult,
                    )

                # ---- attention iterations (pair-major), exp-paced pipeline ----
                LAG = 8
                for it in range(8):
                    p, qcc = divmod(it, 2)
                    hA, hB = 2 * p, 2 * p + 1
                    ex = list(extras[it])
                    es = []  # e tiles: [eA_lo, eB_lo] then [eA_hi, eB_hi]
                    pos = None
                    for s in range(16):
                        if s % 8 == 0:
                            eA = epool.tile([P, 8, 1024], BF, tag="e", name="eA")
                            eB = epool.tile([P, 8, 1024], BF, tag="e", name="eB")
                            es.append((eA, eB))
                        eA, eB = es[s // 8]
                        psA = spool.tile([P, 1024], F32, tag="ps", name="psA")
                        psB = spool.tile([P, 1024], F32, tag="ps", name="psB")
                        for half in range(2):
                            qcol = qcc * 1024 + half * 512
                            nc.tensor.matmul(
                                psA[:, half * 512 : half * 512 + 512],
                                lhsT=kts[p][0:64, s * P : (s + 1) * P],
                                rhs=qps[p][0:64, qcol : qcol + 512],
                                start=True,
                                stop=True,
                                tile_position=(0, 0),
                            )
                            nc.tensor.matmul(
                                psB[:, half * 512 : half * 512 + 512],
                                lhsT=kts[p][64:128, s * P : (s + 1) * P],
                                rhs=qps[p][64:128, qcol : qcol + 512],
                                start=True,
                                stop=True,
                                tile_position=(64, 0),
                            )
                        nc.scalar.activation(eA[:, s % 8, :], psA[:], Exp, scale=SCALE)
                        nc.scalar.activation(eB[:, s % 8, :], psB[:], Exp, scale=SCALE)
                        if ex:
                            ex.pop(0)()
                        if s >= LAG:
                            kt = s - LAG
                            if pos is None:
                                po_A = opool.tile([P, 1024], F32, tag="po", name="poA")
                                po_B = opool.tile([P, 1024], F32, tag="po", name="poB")
                                pos = (po_A, po_B)
                            po_A, po_B = pos
                            elA, elB = es[kt // 8]
                            for half in range(2):
                                hs = slice(half * 512, half * 512 + 512)
                                nc.tensor.matmul(
                                    po_A[0 : HD + 1, hs],
                                    lhsT=V4[:, kt, hA, :],
                                    rhs=elA[:, kt % 8, hs],
                                    start=(kt == 0),
                                    stop=(kt == 15),
                                )
                                nc.tensor.matmul(
                                    po_B[0 : HD + 1, hs],
                                    lhsT=V4[:, kt, hB, :],
                                    rhs=elB[:, kt % 8, hs],
                                    start=(kt == 0),
                                    stop=(kt == 15),
                                )
                    while ex:
                        ex.pop(0)()
                    po_A, po_B = pos
                    for kt in range(16 - LAG, 16):
                        elA, elB = es[kt // 8]
                        for half in range(2):
                            hs = slice(half * 512, half * 512 + 512)
                            nc.tensor.matmul(
                                po_A[0 : HD + 1, hs],
                                lhsT=V4[:, kt, hA, :],
                                rhs=elA[:, kt % 8, hs],
                                start=(kt == 0),
                                stop=(kt == 15),
                            )
                            nc.tensor.matmul(
                                po_B[0 : HD + 1, hs],
                                lhsT=V4[:, kt, hB, :],
                                rhs=elB[:, kt % 8, hs],
                                start=(kt == 0),
                                stop=(kt == 15),
                            )
                    emit_norm(p, qcc, po_A, slice(0, 64))
                    emit_norm(p, qcc, po_B, slice(64, 128))

                ph1_cm.__exit__(None, None, None)

                # ---- projection: yT[e, t] = wpT.T @ AT ----
                with (
                    tc.tile_pool(name="wpool", bufs=1) as wpool,
                    tc.tile_pool(name="outp", bufs=2) as outp,
                ):
                    wps = []
                    for dt in range(4):
                        w_ = wpool.tile([P, D], BF, tag=f"wp{dt}", name=f"wp{dt}")
                        nc.sync.dma_start(w_[:], wpT.ap()[dt * P : (dt + 1) * P, :])
                        wps.append(w_)
                    for et in range(8):
                        for tcc in range(2):
                            pp = spool.tile([P, 1024], F32, tag="ps", name="pp")
                            for half in range(2):
                                col = tcc * 1024 + half * 512
                                for dt in range(4):
                                    nc.tensor.matmul(
                                        pp[:, half * 512 : half * 512 + 512],
                                        lhsT=wps[dt][:, et * P : (et + 1) * P],
                                        rhs=ats[dt][:, col : col + 512],
                                        start=(dt == 0),
                                        stop=(dt == 3),
                                    )
                            for half in range(2):
                                hs = slice(half * 512, half * 512 + 512)
                                ob = outp.tile([P, 512], F32, tag="ob")
                                nc.vector.tensor_scalar_add(
                                    ob[:], pp[:, hs], pb_sb[:, et : et + 1]
                                )
                                nc.sync.dma_start(
                                    yT.ap()[
                                        et * P : (et + 1) * P,
                                        tcc * 1024 + half * 512 : tcc * 1024
                                        + half * 512
                                        + 512,
                                    ],
                                    ob[:],
                                )

                opool_cm.__exit__(None, None, None)
                spool_cm.__exit__(None, None, None)
                npool_cm.__exit__(None, None, None)
                epool_cm.__exit__(None, None, None)

    nc.compile()
    return nc


def kernel(x, qkv_w, qkv_b, proj_w, proj_b):
    global _built, LAST_RESULT
    x = np.asarray(x, np.float32)
    qkv_w = np.asarray(qkv_w, np.float32)
    qkv_b = np.asarray(qkv_b, np.float32)
    proj_w = np.asarray(proj_w, np.float32)
    proj_b = np.asarray(proj_b, np.float32)

    if _built is None:
        _built = _build()
    nc = _built

    in_maps = []
    for c in range(8):
        b, j = divmod(c, 2)
        s = j * 512
        wqkT = np.concatenate([qkv_w[s : s + 512], qkv_w[1024 + s : 1024 + s + 512]]).T
        in_maps.append(
            {
                "xT": np.ascontiguousarray(x[b].T).astype(NBF),
                "wqkT": np.ascontiguousarray(wqkT).astype(NBF),
                "wvT": np.ascontiguousarray(qkv_w[2048 + s : 2048 + s + 512].T).astype(NBF),
                "wpT": np.ascontiguousarray(proj_w[:, s : s + 512].T).astype(NBF),
                "qkb": np.concatenate([qkv_b[s : s + 512], qkv_b[1024 + s : 1024 + s + 512]]),
                "vb": np.ascontiguousarray(qkv_b[2048 + s : 2048 + s + 512]),
                "pb": proj_b if j == 0 else np.zeros_like(proj_b),
            }
        )

    trace = os.environ.get("BASS_TRACE") == "1"
    if trace:
        try:
            import antenv.axon_hooks  # noqa: F401  (needed by the axon trace path)
        except ImportError:
            trace = False
            os.environ["BASS_NEVER_TRACE"] = "1"
    res = run_bass_kernel_spmd(nc, in_maps, core_ids=list(range(8)), trace=trace)
    LAST_RESULT = res

    out = np.empty((B, T, D), np.float32)
    for b in range(B):
        out[b] = (res.results[2 * b]["yT"] + res.results[2 * b + 1]["yT"]).T
    return out
